# revision 14
# baseline (speedup 1.0000x reference)
# ConvLSTM (residual variant) Trainium2 Bass kernel.
#
# Reference computation (N=512, CIN=128, CH=256, spatial 8x8):
#   gates = conv3x3(refpad(x), Wi) + conv3x3(refpad(h0), Wh) + bi + bh
#   i = sigmoid(gi + c0*wc_i); f = sigmoid(gf + c0*wc_f); g = tanh(gg)
#   c_t = c0 + f*c0 + i*g
#   o = sigmoid(go + c_t*wc_o)
#   h_t = h0 + o*tanh(c_t)
#   returns (o, stack([h_t, c_t], axis=1))
#
# Strategy: pure data parallel over 8 NeuronCores (64 samples each).
# The 3x3 conv is computed as 9 shifted accumulating matmuls per
# contraction tile (K=128), reading the reflection-padded input (padded
# host-side) through strided access patterns — no im2col materialization.
# Channels live on SBUF partitions; free dim is (8 samples x 64 out px).

import os

import numpy as np

import concourse.bass as bass
import concourse.mybir as mybir
import concourse.tile as tile
from concourse import bacc
from concourse.bass_utils import run_bass_kernel_spmd

F32 = mybir.dt.float32

N_CORES = 8
N, CIN, CH, W = 512, 128, 256, 8
B = N // N_CORES          # samples per core
S = 8                     # samples per group (free dim = S*64 = 512)
G = B // S                # groups per core
PW = W + 2                # padded height (10)
PWX = 16                  # padded row length: 32B-aligned rows for fp16 PE reads
NGATE = 4
MT = 8                    # out-channel tiles of 128 (4 gates x 256 ch)
NT = 2                    # ch subtiles of CH (256/128)
NKT = 3                   # contraction tiles: 1 for x (CIN), 2 for h (CH)

# env-tunable matmul dtype: "f32" (exact, 4 cyc/row) or "f32r" (1 cyc/row)
MM_DTYPE = os.environ.get("CONVLSTM_MM_DTYPE", "f32r")

LAST_RESULTS = None  # BassKernelResults of the most recent run (for test.py)

_BUILT: dict = {}


def _build(use_wc: bool, mm_dtype: str):
    key = (use_wc, mm_dtype)
    if key in _BUILT:
        return _BUILT[key]

    nc = bacc.Bacc(
        "TRN2",
        target_bir_lowering=False,
        debug=False,
        enable_asserts=False,
        num_devices=N_CORES,
    )

    if mm_dtype == "f32":
        mmdt_decl = F32
    elif mm_dtype == "bf16":
        mmdt_decl = mybir.dt.bfloat16
    elif mm_dtype == "f16":
        mmdt_decl = mybir.dt.float16
    else:
        mmdt_decl = mybir.dt.float32r
    wdt = mmdt_decl
    xpad_d = nc.dram_tensor("xpad", [CIN, B, PW, PWX], mmdt_decl,
                            kind="ExternalInput")
    hpad_d = nc.dram_tensor("hpad", [NT, 128, B, PW, PWX], mmdt_decl,
                            kind="ExternalInput")
    c0_d = nc.dram_tensor("c0", [NT, 128, B, W * W], F32, kind="ExternalInput")
    h0_d = nc.dram_tensor("h0", [NT, 128, B, W * W], F32, kind="ExternalInput")
    wts_d = nc.dram_tensor("wts", [NT, 128, NKT, 9, NGATE * 128], wdt,
                           kind="ExternalInput")
    bias_d = nc.dram_tensor("bias", [128, MT], F32, kind="ExternalInput")
    if use_wc:
        wcb_d = nc.dram_tensor("wcb", [128, 3, NT, S, W * W], F32, kind="ExternalInput")
    out_d = nc.dram_tensor(
        "out", [NT, 128, G, 3, S, W * W], F32, kind="ExternalOutput"
    )

    mmdt = mmdt_decl

    with tile.TileContext(nc) as tc:
        with (
            tc.tile_pool(name="singles", bufs=1) as singles,
            tc.tile_pool(name="stream", bufs=2) as stream,
            tc.tile_pool(name="tmps", bufs=2) as tmps,
            tc.tile_pool(name="stage", bufs=2) as stagep,
            tc.tile_pool(name="psum", bufs=2, space=bass.MemorySpace.PSUM) as psump,
        ):
            bias_sb = singles.tile([128, MT], F32)
            nc.gpsimd.dma_start(out=bias_sb[:], in_=bias_d[:])
            w_tk = [[singles.tile([128, 9, NGATE * 128], wdt,
                                  name=f"w_{t}_{kt}")
                     for kt in range(NKT)] for t in range(NT)]
            if use_wc:
                wcb_sb = singles.tile([128, 3, NT, S, W * W], F32)
                nc.gpsimd.dma_start(out=wcb_sb[:], in_=wcb_d[:])

            for g in range(G):
                s0 = g * S
                x_t = stream.tile([128, S, PW, PWX], mmdt, tag="x")
                x_eng = nc.gpsimd if g == 0 else nc.sync
                x_eng.dma_start(out=x_t[:], in_=xpad_d[:, s0 : s0 + S])
                h_t = []
                for kt in range(NT):
                    ht = stream.tile([128, S, PW, PWX], mmdt, tag=f"h{kt}")
                    nc.scalar.dma_start(out=ht[:], in_=hpad_d[kt, :, s0 : s0 + S])
                    h_t.append(ht)
                if g == 0:
                    # weight slabs ordered so the first matmuls wait only on
                    # x + the first chunk of w[t=0][kt=0]; first slab split
                    # across the gpsimd (early-issue) and sync rings
                    nc.gpsimd.dma_start(out=w_tk[0][0][:, 0:5],
                                        in_=wts_d[0, :, 0, 0:5])
                    nc.sync.dma_start(out=w_tk[0][0][:, 5:9],
                                      in_=wts_d[0, :, 0, 5:9])
                    for t in range(NT):
                        for kt in range(NKT):
                            if t == 0 and kt == 0:
                                continue
                            nc.gpsimd.dma_start(out=w_tk[t][kt][:],
                                                in_=wts_d[t, :, kt])
                c0_t = []
                h0_t = []
                for kt in range(NT):
                    ct = stream.tile([128, S, W * W], F32, tag=f"c{kt}")
                    nc.gpsimd.dma_start(out=ct[:], in_=c0_d[kt, :, s0 : s0 + S])
                    c0_t.append(ct)
                    h0t = stream.tile([128, S, W * W], F32, tag=f"hh{kt}")
                    nc.gpsimd.dma_start(out=h0t[:], in_=h0_d[kt, :, s0 : s0 + S])
                    h0_t.append(h0t)

                for t in range(NT):
                    # ---- conv: accumulate 4 gate tiles of [128, S*64];
                    # kt-outer so the first 36 matmuls need only x + w_kt[0]
                    ps = [psump.tile([128, S, W * W], F32, tag=f"ps{gate}",
                                     name=f"ps{gate}_{g}_{t}")
                          for gate in range(NGATE)]
                    for kt in range(NKT):
                        src = x_t if kt == 0 else h_t[kt - 1]
                        for ky in range(3):
                            for kx in range(3):
                                rhs = src[:, :, ky : ky + W, kx : kx + W]
                                for gate in range(NGATE):
                                    lhsT = w_tk[t][kt][:, ky * 3 + kx,
                                                       gate * 128 :
                                                       (gate + 1) * 128]
                                    nc.tensor.matmul(
                                        ps[gate][:],
                                        lhsT,
                                        rhs,
                                        start=(kt == 0 and ky == 0 and kx == 0),
                                        stop=(kt == NKT - 1 and ky == 2
                                              and kx == 2),
                                    )

                    # ---- LSTM epilogue ----
                    Sig = mybir.ActivationFunctionType.Sigmoid
                    Tanh = mybir.ActivationFunctionType.Tanh
                    Add = mybir.AluOpType.add
                    Mult = mybir.AluOpType.mult

                    def b_ap(gate):
                        m = gate * 2 + t
                        return bias_sb[:, m : m + 1]

                    stage = stagep.tile([128, 3, S, W * W], F32, tag="stage")
                    c0t = c0_t[t]
                    h0v = h0_t[t][:]

                    def gate_in(gate, which):
                        # returns AP to feed activation for gate `gate`
                        if not use_wc:
                            return ps[gate][:]
                        u1 = tmps.tile([128, S, W * W], F32, tag="u1")
                        src = c0t[:] if gate != 3 else stage[:, 2]
                        nc.vector.tensor_mul(u1[:], src, wcb_sb[:, which, t])
                        u2 = tmps.tile([128, S, W * W], F32, tag="u2")
                        nc.vector.tensor_add(u2[:], u1[:], ps[gate][:])
                        return u2[:]

                    i_s = tmps.tile([128, S, W * W], F32, tag="i")
                    nc.scalar.activation(i_s[:], gate_in(0, 0), Sig, bias=b_ap(0))
                    f_s = tmps.tile([128, S, W * W], F32, tag="f")
                    nc.scalar.activation(f_s[:], gate_in(1, 1), Sig, bias=b_ap(1))
                    g_s = tmps.tile([128, S, W * W], F32, tag="g")
                    nc.scalar.activation(g_s[:], ps[2][:], Tanh, bias=b_ap(2))

                    # c_t = (f+1)*c0 + i*g
                    t1 = tmps.tile([128, S, W * W], F32, tag="t1")
                    nc.vector.scalar_tensor_tensor(
                        t1[:], f_s[:], 1.0, c0t[:], op0=Add, op1=Mult
                    )
                    t2 = tmps.tile([128, S, W * W], F32, tag="t2")
                    nc.vector.tensor_mul(t2[:], i_s[:], g_s[:])
                    nc.vector.tensor_add(stage[:, 2], t1[:], t2[:])

                    # o = sigmoid(go + c_t*wc_o + b_o)
                    nc.scalar.activation(stage[:, 0], gate_in(3, 2), Sig, bias=b_ap(3))

                    # h_t = h0 + o*tanh(c_t)
                    th = tmps.tile([128, S, W * W], F32, tag="th")
                    nc.scalar.activation(th[:], stage[:, 2], Tanh)
                    t3 = tmps.tile([128, S, W * W], F32, tag="t3")
                    nc.vector.tensor_mul(t3[:], stage[:, 0], th[:])
                    nc.vector.tensor_add(stage[:, 1], t3[:], h0v)

                    nc.sync.dma_start(out=out_d[t, :, g], in_=stage[:])

    nc.compile()
    _BUILT[key] = nc
    return nc


def _refpad(a):
    # reflection-pad the last two (8x8) dims to 10x10, then zero-pad the
    # row length to PWX so each row is 32B-aligned in SBUF
    p = np.pad(a, ((0, 0), (0, 0), (1, 1), (1, 1)), mode="reflect")
    out = np.zeros(p.shape[:-1] + (PWX,), p.dtype)
    out[..., :PW] = p
    return out


def _prepare(x, hidden_state, w_ii, b_ii, w_if, b_if, w_ig, b_ig, w_io, b_io,
             w_hi, b_hi, w_hf, b_hf, w_hg, b_hg, w_ho, b_ho, wc_i, wc_f, wc_o):
    f = np.float32
    x = np.ascontiguousarray(np.asarray(x, f))
    hs = np.ascontiguousarray(np.asarray(hidden_state, f))
    wc = [np.asarray(a, f).reshape(CH, W * W) for a in (wc_i, wc_f, wc_o)]
    use_wc = any(np.any(a) for a in wc)

    # ---- host-side layout prep ----
    # x: (N, CIN, 8, 8) -> pad -> (CIN, N, 10, 10)
    xp = _refpad(x).transpose(1, 0, 2, 3)
    xp = np.ascontiguousarray(xp)
    # h0: (N, CH, 8, 8) -> pad -> (NT, 128, N, 10, 10)
    hp = _refpad(hs[:, 0]).transpose(1, 0, 2, 3).reshape(NT, 128, N, PW, PWX)
    hp = np.ascontiguousarray(hp)
    # c0/h0: (N, CH, 8, 8) -> (NT, 128, N, 64)
    c0 = hs[:, 1].transpose(1, 0, 2, 3).reshape(CH, N, W * W)
    c0 = np.ascontiguousarray(c0.reshape(NT, 128, N, W * W))
    h0 = hs[:, 0].transpose(1, 0, 2, 3).reshape(CH, N, W * W)
    h0 = np.ascontiguousarray(h0.reshape(NT, 128, N, W * W))

    # weights: lhsT[k, m] per (ky, kx, ktile); m = gate*CH + och
    Wi = np.concatenate([np.asarray(a, f) for a in (w_ii, w_if, w_ig, w_io)], 0)
    Wh = np.concatenate([np.asarray(a, f) for a in (w_hi, w_hf, w_hg, w_ho)], 0)
    # (M, K, 3, 3) -> (3, 3, K, M)
    wi_t = Wi.transpose(2, 3, 1, 0)               # (3,3,128,1024)
    wh_t = Wh.transpose(2, 3, 1, 0)               # (3,3,256,1024)
    wts0 = np.empty((128, NKT, 9, NGATE * CH), f)
    for ky in range(3):
        for kx in range(3):
            o9 = ky * 3 + kx
            wts0[:, 0, o9] = wi_t[ky, kx]
            wts0[:, 1, o9] = wh_t[ky, kx, :128]
            wts0[:, 2, o9] = wh_t[ky, kx, 128:]
    # split out-channel columns t-major: [t, p, kt, off, gate*128+c]
    wts = np.ascontiguousarray(
        wts0.reshape(128, NKT, 9, NGATE, NT, 128)
        .transpose(4, 0, 1, 2, 3, 5)
        .reshape(NT, 128, NKT, 9, NGATE * 128))

    bt = (np.concatenate([np.asarray(a, f) for a in (b_ii, b_if, b_ig, b_io)])
          + np.concatenate([np.asarray(a, f) for a in (b_hi, b_hf, b_hg, b_ho)]))
    bias = np.ascontiguousarray(bt.reshape(NGATE, NT, 128).transpose(2, 0, 1)
                                .reshape(128, MT))

    in_maps = []
    for c in range(N_CORES):
        sl = slice(c * B, (c + 1) * B)
        m = {
            "xpad": np.ascontiguousarray(xp[:, sl]),
            "hpad": np.ascontiguousarray(hp[:, :, sl]),
            "c0": np.ascontiguousarray(c0[:, :, sl]),
            "h0": np.ascontiguousarray(h0[:, :, sl]),
            "wts": wts,
            "bias": bias,
        }
        if MM_DTYPE in ("bf16", "f16"):
            import ml_dtypes
            bf = ml_dtypes.bfloat16 if MM_DTYPE == "bf16" else np.float16
            for k in ("xpad", "hpad", "wts"):
                m[k] = np.ascontiguousarray(m[k].astype(bf))
        if use_wc:
            wcb = np.stack(wc).reshape(3, NT, 128, W * W)           # (3,NT,128,64)
            wcb = np.broadcast_to(wcb[:, :, :, None, :], (3, NT, 128, S, W * W))
            m["wcb"] = np.ascontiguousarray(
                wcb.transpose(2, 0, 1, 3, 4))                        # (128,3,NT,S,64)
        in_maps.append(m)
    return in_maps, use_wc


def _gather(core_outs):
    f = np.float32
    o_full = np.empty((N, CH, W, W), f)
    h_full = np.empty((N, CH, W, W), f)
    c_full = np.empty((N, CH, W, W), f)
    for c, arr in enumerate(core_outs):                 # (NT,128,G,3,S,64)
        # -> (3, G, S, NT, 128, 64) -> (3, B, CH, 8, 8)
        arr = arr.transpose(3, 2, 4, 0, 1, 5).reshape(3, B, CH, W, W)
        sl = slice(c * B, (c + 1) * B)
        o_full[sl] = arr[0]
        h_full[sl] = arr[1]
        c_full[sl] = arr[2]
    hidden_out = np.stack([h_full, c_full], axis=1)
    return o_full, hidden_out


def _install_ntff_hook():
    """Recreate antenv.axon_hooks (absent on this image) so trace=True works.

    Mirrors trn_boot._ntff_profile_via_ctypes against /opt/axon/libaxon_pjrt.so.
    Only used by our own test harness (CONVLSTM_TRACE=1), never in grading.
    """
    import sys
    import types
    import ctypes
    import contextlib

    if "antenv.axon_hooks" in sys.modules:
        return
    lib = ctypes.CDLL("/opt/axon/libaxon_pjrt.so")
    lib.axon_start_nrt_profile.argtypes = [
        ctypes.POINTER(ctypes.c_int64), ctypes.c_size_t]
    lib.axon_start_nrt_profile.restype = ctypes.c_int64
    lib.axon_stop_nrt_profile.argtypes = [ctypes.c_char_p]
    lib.axon_stop_nrt_profile.restype = ctypes.c_int64

    @contextlib.contextmanager
    def _hook(output_dir, device_ids):
        import jax
        jax.devices()
        if device_ids:
            ids = (ctypes.c_int64 * len(device_ids))(*device_ids)
            rc = lib.axon_start_nrt_profile(ids, len(device_ids))
        else:
            rc = lib.axon_start_nrt_profile(None, 0)
        if rc != 0:
            raise RuntimeError(f"axon_start_nrt_profile rc={rc}")
        try:
            yield
        finally:
            n = lib.axon_stop_nrt_profile(str(output_dir).encode())
            print(f"ntff profile: {n} file(s) written to {output_dir}")

    mod = types.ModuleType("antenv.axon_hooks")
    mod.get_axon_ntff_profile_hook = lambda: _hook
    mod.set_axon_ntff_profile_hook = lambda h: None
    sys.modules["antenv.axon_hooks"] = mod

    # artifact upload has no bucket in this container; make it a no-op
    import concourse.bass_utils as bu
    bu.upload_artifacts = lambda tmpdir: ""


def kernel(**inputs):
    global LAST_RESULTS
    in_maps, use_wc = _prepare(**inputs)
    nc = _build(use_wc, MM_DTYPE)
    trace = os.environ.get("CONVLSTM_TRACE", "0") == "1"
    if trace:
        _install_ntff_hook()
    res = run_bass_kernel_spmd(
        nc, in_maps, core_ids=list(range(N_CORES)), trace=trace,
        tmpdir=os.environ.get("CONVLSTM_TRACE_DIR") or None,
    )
    LAST_RESULTS = res
    return _gather([r["out"] for r in res.results])


# revision 16
# speedup vs baseline: 1.1673x; 1.1673x over previous
# ConvLSTM (residual variant) Trainium2 Bass kernel.
#
# Reference computation (N=512, CIN=128, CH=256, spatial 8x8):
#   gates = conv3x3(refpad(x), Wi) + conv3x3(refpad(h0), Wh) + bi + bh
#   i = sigmoid(gi + c0*wc_i); f = sigmoid(gf + c0*wc_f); g = tanh(gg)
#   c_t = c0 + f*c0 + i*g
#   o = sigmoid(go + c_t*wc_o)
#   h_t = h0 + o*tanh(c_t)
#   returns (o, stack([h_t, c_t], axis=1))
#
# Strategy: pure data parallel over 8 NeuronCores (64 samples each).
# The 3x3 conv is computed as 9 shifted accumulating matmuls per
# contraction tile (K=128), reading the reflection-padded input (padded
# host-side) through strided access patterns — no im2col materialization.
# Channels live on SBUF partitions; free dim is (8 samples x 64 out px).

import os

import numpy as np

import concourse.bass as bass
import concourse.mybir as mybir
import concourse.tile as tile
from concourse import bacc
from concourse.bass_utils import run_bass_kernel_spmd

F32 = mybir.dt.float32

N_CORES = 8
N, CIN, CH, W = 512, 128, 256, 8
B = N // N_CORES          # samples per core
S = 8                     # samples per group (free dim = S*64 = 512)
G = B // S                # groups per core
PW = W + 2                # padded height/width (10)
NGATE = 4
MT = 8                    # out-channel tiles of 128 (4 gates x 256 ch)
NT = 2                    # ch subtiles of CH (256/128)
NKT = 3                   # contraction tiles: 1 for x (CIN), 2 for h (CH)

# env-tunable matmul dtype: "f32" (exact, 4 cyc/row) or "f32r" (1 cyc/row)
MM_DTYPE = os.environ.get("CONVLSTM_MM_DTYPE", "f16")

LAST_RESULTS = None  # BassKernelResults of the most recent run (for test.py)

_BUILT: dict = {}


def _build(use_wc: bool, mm_dtype: str):
    key = (use_wc, mm_dtype)
    if key in _BUILT:
        return _BUILT[key]

    nc = bacc.Bacc(
        "TRN2",
        target_bir_lowering=False,
        debug=False,
        enable_asserts=False,
        num_devices=N_CORES,
    )

    if mm_dtype == "f32":
        mmdt_decl = F32
    elif mm_dtype == "bf16":
        mmdt_decl = mybir.dt.bfloat16
    elif mm_dtype == "f16":
        mmdt_decl = mybir.dt.float16
    else:
        mmdt_decl = mybir.dt.float32r
    wdt = mmdt_decl
    # samples innermost: per group the free layout is (row, col, sample),
    # so each matmul reads 8 contiguous (col,sample)=64-elem runs
    xpad_d = nc.dram_tensor("xpad", [CIN, G, PW, PW, S], mmdt_decl,
                            kind="ExternalInput")
    hpad_d = nc.dram_tensor("hpad", [NT, 128, G, PW, PW, S], mmdt_decl,
                            kind="ExternalInput")
    c0_d = nc.dram_tensor("c0", [NT, 128, G, W, W, S], F32, kind="ExternalInput")
    h0_d = nc.dram_tensor("h0", [NT, 128, G, W, W, S], F32, kind="ExternalInput")
    wts_d = nc.dram_tensor("wts", [NT, 128, NKT, 9, NGATE * 128], wdt,
                           kind="ExternalInput")
    bias_d = nc.dram_tensor("bias", [128, MT], F32, kind="ExternalInput")
    if use_wc:
        wcb_d = nc.dram_tensor("wcb", [128, 3, NT, W, W, S], F32,
                               kind="ExternalInput")
    out_d = nc.dram_tensor(
        "out", [NT, 128, G, 3, W, W, S], F32, kind="ExternalOutput"
    )

    mmdt = mmdt_decl

    with tile.TileContext(nc) as tc:
        with (
            tc.tile_pool(name="singles", bufs=1) as singles,
            tc.tile_pool(name="stream", bufs=2) as stream,
            tc.tile_pool(name="tmps", bufs=2) as tmps,
            tc.tile_pool(name="stage", bufs=2) as stagep,
            tc.tile_pool(name="psum", bufs=2, space=bass.MemorySpace.PSUM) as psump,
        ):
            bias_sb = singles.tile([128, MT], F32)
            nc.gpsimd.dma_start(out=bias_sb[:], in_=bias_d[:])
            w_tk = [[singles.tile([128, 9, NGATE * 128], wdt,
                                  name=f"w_{t}_{kt}")
                     for kt in range(NKT)] for t in range(NT)]
            if use_wc:
                wcb_sb = singles.tile([128, 3, NT, S, W * W], F32)
                nc.gpsimd.dma_start(out=wcb_sb[:], in_=wcb_d[:])

            for g in range(G):
                s0 = g * S
                x_t = stream.tile([128, PW, PW, S], mmdt, tag="x")
                x_eng = nc.gpsimd if g == 0 else nc.sync
                x_eng.dma_start(out=x_t[:], in_=xpad_d[:, g])
                h_t = []
                for kt in range(NT):
                    ht = stream.tile([128, PW, PW, S], mmdt, tag=f"h{kt}")
                    nc.scalar.dma_start(out=ht[:], in_=hpad_d[kt, :, g])
                    h_t.append(ht)
                if g == 0:
                    # weight slabs ordered so the first matmuls wait only on
                    # x + the first chunk of w[t=0][kt=0]; first slab split
                    # across the gpsimd (early-issue) and sync rings
                    nc.gpsimd.dma_start(out=w_tk[0][0][:, 0:5],
                                        in_=wts_d[0, :, 0, 0:5])
                    nc.sync.dma_start(out=w_tk[0][0][:, 5:9],
                                      in_=wts_d[0, :, 0, 5:9])
                    for t in range(NT):
                        for kt in range(NKT):
                            if t == 0 and kt == 0:
                                continue
                            nc.gpsimd.dma_start(out=w_tk[t][kt][:],
                                                in_=wts_d[t, :, kt])
                c0_t = []
                h0_t = []
                for kt in range(NT):
                    ct = stream.tile([128, W, W, S], F32, tag=f"c{kt}")
                    nc.gpsimd.dma_start(out=ct[:], in_=c0_d[kt, :, g])
                    c0_t.append(ct)
                    h0t = stream.tile([128, W, W, S], F32, tag=f"hh{kt}")
                    nc.gpsimd.dma_start(out=h0t[:], in_=h0_d[kt, :, g])
                    h0_t.append(h0t)

                for t in range(NT):
                    # ---- conv: accumulate 4 gate tiles of [128, S*64];
                    # kt-outer so the first 36 matmuls need only x + w_kt[0]
                    ps = [psump.tile([128, W, W, S], F32, tag=f"ps{gate}",
                                     name=f"ps{gate}_{g}_{t}")
                          for gate in range(NGATE)]
                    for kt in range(NKT):
                        src = x_t if kt == 0 else h_t[kt - 1]
                        for ky in range(3):
                            for kx in range(3):
                                rhs = src[:, ky : ky + W, kx : kx + W, :]
                                for gate in range(NGATE):
                                    lhsT = w_tk[t][kt][:, ky * 3 + kx,
                                                       gate * 128 :
                                                       (gate + 1) * 128]
                                    nc.tensor.matmul(
                                        ps[gate][:],
                                        lhsT,
                                        rhs,
                                        start=(kt == 0 and ky == 0 and kx == 0),
                                        stop=(kt == NKT - 1 and ky == 2
                                              and kx == 2),
                                    )

                    # ---- LSTM epilogue ----
                    Sig = mybir.ActivationFunctionType.Sigmoid
                    Tanh = mybir.ActivationFunctionType.Tanh
                    Add = mybir.AluOpType.add
                    Mult = mybir.AluOpType.mult

                    def b_ap(gate):
                        m = gate * 2 + t
                        return bias_sb[:, m : m + 1]

                    stage = stagep.tile([128, 3, W, W, S], F32, tag="stage")
                    c0t = c0_t[t]
                    h0v = h0_t[t][:]

                    def gate_in(gate, which):
                        # returns AP to feed activation for gate `gate`
                        if not use_wc:
                            return ps[gate][:]
                        u1 = tmps.tile([128, W, W, S], F32, tag="u1")
                        src = c0t[:] if gate != 3 else stage[:, 2]
                        nc.vector.tensor_mul(u1[:], src, wcb_sb[:, which, t])
                        u2 = tmps.tile([128, W, W, S], F32, tag="u2")
                        nc.vector.tensor_add(u2[:], u1[:], ps[gate][:])
                        return u2[:]

                    i_s = tmps.tile([128, W, W, S], F32, tag="i")
                    nc.scalar.activation(i_s[:], gate_in(0, 0), Sig, bias=b_ap(0))
                    f_s = tmps.tile([128, W, W, S], F32, tag="f")
                    nc.scalar.activation(f_s[:], gate_in(1, 1), Sig, bias=b_ap(1))
                    g_s = tmps.tile([128, W, W, S], F32, tag="g")
                    nc.scalar.activation(g_s[:], ps[2][:], Tanh, bias=b_ap(2))

                    # c_t = (f+1)*c0 + i*g
                    t1 = tmps.tile([128, W, W, S], F32, tag="t1")
                    nc.vector.scalar_tensor_tensor(
                        t1[:], f_s[:], 1.0, c0t[:], op0=Add, op1=Mult
                    )
                    t2 = tmps.tile([128, W, W, S], F32, tag="t2")
                    nc.vector.tensor_mul(t2[:], i_s[:], g_s[:])
                    nc.vector.tensor_add(stage[:, 2], t1[:], t2[:])

                    # o = sigmoid(go + c_t*wc_o + b_o)
                    nc.scalar.activation(stage[:, 0], gate_in(3, 2), Sig, bias=b_ap(3))

                    # h_t = h0 + o*tanh(c_t)
                    th = tmps.tile([128, W, W, S], F32, tag="th")
                    nc.scalar.activation(th[:], stage[:, 2], Tanh)
                    t3 = tmps.tile([128, W, W, S], F32, tag="t3")
                    nc.vector.tensor_mul(t3[:], stage[:, 0], th[:])
                    nc.vector.tensor_add(stage[:, 1], t3[:], h0v)

                    nc.sync.dma_start(out=out_d[t, :, g], in_=stage[:])

    nc.compile()
    _BUILT[key] = nc
    return nc


def _refpad(a):
    # reflection-pad the last two (8x8) dims to 10x10
    return np.pad(a, ((0, 0), (0, 0), (1, 1), (1, 1)), mode="reflect")


def _prepare(x, hidden_state, w_ii, b_ii, w_if, b_if, w_ig, b_ig, w_io, b_io,
             w_hi, b_hi, w_hf, b_hf, w_hg, b_hg, w_ho, b_ho, wc_i, wc_f, wc_o):
    f = np.float32
    x = np.ascontiguousarray(np.asarray(x, f))
    hs = np.ascontiguousarray(np.asarray(hidden_state, f))
    wc = [np.asarray(a, f).reshape(CH, W * W) for a in (wc_i, wc_f, wc_o)]
    use_wc = any(np.any(a) for a in wc)

    # ---- host-side layout prep ----
    # x: (N, CIN, 8, 8) -> pad -> (CIN, N//S groups, 10, 10, S) samples-inner
    def to_groups(a, nch):
        # (N, nch, py, px) -> (nch, N//S, py, px, S)
        py, px = a.shape[2], a.shape[3]
        a = a.reshape(N // S, S, nch, py, px)
        return np.ascontiguousarray(a.transpose(2, 0, 3, 4, 1))

    xp = to_groups(_refpad(x), CIN)                        # (128, N/S, 10, 10, S)
    hp = to_groups(_refpad(hs[:, 0]), CH).reshape(NT, 128, N // S, PW, PW, S)
    c0 = to_groups(hs[:, 1], CH).reshape(NT, 128, N // S, W, W, S)
    h0 = to_groups(hs[:, 0], CH).reshape(NT, 128, N // S, W, W, S)

    # weights: lhsT[k, m] per (ky, kx, ktile); m = gate*CH + och
    Wi = np.concatenate([np.asarray(a, f) for a in (w_ii, w_if, w_ig, w_io)], 0)
    Wh = np.concatenate([np.asarray(a, f) for a in (w_hi, w_hf, w_hg, w_ho)], 0)
    # (M, K, 3, 3) -> (3, 3, K, M)
    wi_t = Wi.transpose(2, 3, 1, 0)               # (3,3,128,1024)
    wh_t = Wh.transpose(2, 3, 1, 0)               # (3,3,256,1024)
    wts0 = np.empty((128, NKT, 9, NGATE * CH), f)
    for ky in range(3):
        for kx in range(3):
            o9 = ky * 3 + kx
            wts0[:, 0, o9] = wi_t[ky, kx]
            wts0[:, 1, o9] = wh_t[ky, kx, :128]
            wts0[:, 2, o9] = wh_t[ky, kx, 128:]
    # split out-channel columns t-major: [t, p, kt, off, gate*128+c]
    wts = np.ascontiguousarray(
        wts0.reshape(128, NKT, 9, NGATE, NT, 128)
        .transpose(4, 0, 1, 2, 3, 5)
        .reshape(NT, 128, NKT, 9, NGATE * 128))

    bt = (np.concatenate([np.asarray(a, f) for a in (b_ii, b_if, b_ig, b_io)])
          + np.concatenate([np.asarray(a, f) for a in (b_hi, b_hf, b_hg, b_ho)]))
    bias = np.ascontiguousarray(bt.reshape(NGATE, NT, 128).transpose(2, 0, 1)
                                .reshape(128, MT))

    in_maps = []
    for c in range(N_CORES):
        sl = slice(c * G, (c + 1) * G)
        m = {
            "xpad": np.ascontiguousarray(xp[:, sl]),
            "hpad": np.ascontiguousarray(hp[:, :, sl]),
            "c0": np.ascontiguousarray(c0[:, :, sl]),
            "h0": np.ascontiguousarray(h0[:, :, sl]),
            "wts": wts,
            "bias": bias,
        }
        if MM_DTYPE in ("bf16", "f16"):
            import ml_dtypes
            bf = ml_dtypes.bfloat16 if MM_DTYPE == "bf16" else np.float16
            for k in ("xpad", "hpad", "wts"):
                m[k] = np.ascontiguousarray(m[k].astype(bf))
        if use_wc:
            wcb = np.stack(wc).reshape(3, NT, 128, W, W)
            wcb = np.broadcast_to(wcb[..., None], (3, NT, 128, W, W, S))
            m["wcb"] = np.ascontiguousarray(wcb.transpose(2, 0, 1, 3, 4, 5))
        in_maps.append(m)
    return in_maps, use_wc


def _gather(core_outs):
    f = np.float32
    o_full = np.empty((N, CH, W, W), f)
    h_full = np.empty((N, CH, W, W), f)
    c_full = np.empty((N, CH, W, W), f)
    for c, arr in enumerate(core_outs):                 # (NT,128,G,3,W,W,S)
        # -> (3, G, S, NT, 128, W, W) -> (3, B, CH, 8, 8)
        arr = arr.transpose(3, 2, 6, 0, 1, 4, 5).reshape(3, B, CH, W, W)
        sl = slice(c * B, (c + 1) * B)
        o_full[sl] = arr[0]
        h_full[sl] = arr[1]
        c_full[sl] = arr[2]
    hidden_out = np.stack([h_full, c_full], axis=1)
    return o_full, hidden_out


def _install_ntff_hook():
    """Recreate antenv.axon_hooks (absent on this image) so trace=True works.

    Mirrors trn_boot._ntff_profile_via_ctypes against /opt/axon/libaxon_pjrt.so.
    Only used by our own test harness (CONVLSTM_TRACE=1), never in grading.
    """
    import sys
    import types
    import ctypes
    import contextlib

    if "antenv.axon_hooks" in sys.modules:
        return
    lib = ctypes.CDLL("/opt/axon/libaxon_pjrt.so")
    lib.axon_start_nrt_profile.argtypes = [
        ctypes.POINTER(ctypes.c_int64), ctypes.c_size_t]
    lib.axon_start_nrt_profile.restype = ctypes.c_int64
    lib.axon_stop_nrt_profile.argtypes = [ctypes.c_char_p]
    lib.axon_stop_nrt_profile.restype = ctypes.c_int64

    @contextlib.contextmanager
    def _hook(output_dir, device_ids):
        import jax
        jax.devices()
        if device_ids:
            ids = (ctypes.c_int64 * len(device_ids))(*device_ids)
            rc = lib.axon_start_nrt_profile(ids, len(device_ids))
        else:
            rc = lib.axon_start_nrt_profile(None, 0)
        if rc != 0:
            raise RuntimeError(f"axon_start_nrt_profile rc={rc}")
        try:
            yield
        finally:
            n = lib.axon_stop_nrt_profile(str(output_dir).encode())
            print(f"ntff profile: {n} file(s) written to {output_dir}")

    mod = types.ModuleType("antenv.axon_hooks")
    mod.get_axon_ntff_profile_hook = lambda: _hook
    mod.set_axon_ntff_profile_hook = lambda h: None
    sys.modules["antenv.axon_hooks"] = mod

    # artifact upload has no bucket in this container; make it a no-op
    import concourse.bass_utils as bu
    bu.upload_artifacts = lambda tmpdir: ""


def kernel(**inputs):
    global LAST_RESULTS
    in_maps, use_wc = _prepare(**inputs)
    nc = _build(use_wc, MM_DTYPE)
    trace = os.environ.get("CONVLSTM_TRACE", "0") == "1"
    if trace:
        _install_ntff_hook()
    res = run_bass_kernel_spmd(
        nc, in_maps, core_ids=list(range(N_CORES)), trace=trace,
        tmpdir=os.environ.get("CONVLSTM_TRACE_DIR") or None,
    )
    LAST_RESULTS = res
    return _gather([r["out"] for r in res.results])
